# revision 22
# baseline (speedup 1.0000x reference)
"""DistMult edge scoring on 8 Trainium2 NeuronCores.

score[e] = sigmoid(sum_d h[u[e],d] * rel_weight[etype[e],d] * h[v[e],d])

Strategy
--------
Edges are sharded evenly across the 8 cores (pure edge parallelism); h and
rel_weight are replicated per core. The dominant cost is gathering h[u] and
h[v] rows (2 x 250k x D floats), so the kernel is built around the fast
Q7-ucode `dma_gather` (InstDMAGatherAnt):

- int16 gather indices only address 32768 rows, so h is viewed as 4 windows
  of 32768 rows and edges are bucketed by the window pair (u>>15, v>>15).
  Indices are window-relative; each gather instruction reads from its
  window's base AP.
- Every bucket is split evenly across the 8 cores (same per-bucket capacity
  on every core -> one shared SPMD program). Capacities depend on the input,
  so the program is JIT-built per capacity signature and cached.
- 4 SWDGE queues round-robin the gather instructions across the 4 Q7 cpu
  pairs (descriptor generation is the gather bottleneck at ~8.4 ns/row/queue).
- Per gather chunk DVE computes hu*hv for all tiles in one wide fp16
  2x-mode multiply; per 128-edge tile PE expands rel_weight[etype] via a
  one-hot matmul into PSUM and DVE multiplies it in; the free-axis fp32
  reduction is split between DVE (tensor_reduce) and ACT (activation
  accum_out) to balance the engines; ACT applies the sigmoid once at the
  end.
- Tensor data is gathered/multiplied in fp16 (fp32 accumulation): halves the
  gather bytes and doubles DVE throughput at ~1e-3 worst-case relative error.
  Set DTYPE = "float32" for exact mode.
"""

import numpy as np

import concourse.bacc as bacc
import concourse.mybir as mybir
import concourse.tile as tile
from concourse.bass_utils import run_bass_kernel_spmd

N_NODES = 100000
D = 384
N_ETYPES = 8
N_CORES = 8

P = 128
W = 32768                 # int16-addressable h window (rows)
NW = (N_NODES + W - 1) // W   # 4 windows
NB = NW * NW              # 16 (wu, wv) buckets
CH = 1024                 # max gather indices per dma_gather instruction
NQ = 4                    # SWDGE queues

DTYPE = "float16"         # compute/gather dtype: "float16" or "float32"

_cache = {}


def _np_dt():
    return np.float16 if DTYPE == "float16" else np.float32


def _mb_dt():
    return mybir.dt.float16 if DTYPE == "float16" else mybir.dt.float32


def _chunks(cap):
    """Split a bucket capacity (multiple of 128) into gather chunk sizes."""
    out = []
    while cap > 0:
        n = min(cap, CH)
        out.append(n)
        cap -= n
    return out


def _build(caps):
    """Build + compile the SPMD program for per-bucket capacities `caps`
    (tuple of NB ints, each a multiple of 128)."""
    dt = _mb_dt()
    f32 = mybir.dt.float32
    t_tot = sum(caps) // P
    ucols = sum(caps) // 16   # total int16 index columns per side

    nc = bacc.Bacc(
        "TRN2",
        target_bir_lowering=False,
        debug=False,
        enable_asserts=False,
        num_devices=N_CORES,
        num_swdge_queues=NQ,
    )
    h_ap = nc.dram_tensor("h", [N_NODES, D], dt, kind="ExternalInput").ap()
    uidx = nc.dram_tensor("uidx", [P, ucols], mybir.dt.int16, kind="ExternalInput").ap()
    vidx = nc.dram_tensor("vidx", [P, ucols], mybir.dt.int16, kind="ExternalInput").ap()
    oneh = nc.dram_tensor("oneh", [N_ETYPES, t_tot * P], dt, kind="ExternalInput").ap()
    relw = nc.dram_tensor("relw", [N_ETYPES, D], dt, kind="ExternalInput").ap()
    out = nc.dram_tensor("out", [P, t_tot], f32, kind="ExternalOutput").ap()

    q = 0
    with tile.TileContext(nc) as tc:
        with (
            tc.tile_pool(name="const", bufs=1) as cpool,
            tc.tile_pool(name="gath", bufs=6) as gpool,
            tc.tile_pool(name="work", bufs=8) as wpool,
            tc.tile_pool(name="work4", bufs=4) as w4pool,
            tc.tile_pool(name="psum", bufs=6, space="PSUM") as ppool,
        ):
            u_sb = cpool.tile([P, ucols], mybir.dt.int16)
            nc.sync.dma_start(out=u_sb[:], in_=uidx[:])
            v_sb = cpool.tile([P, ucols], mybir.dt.int16)
            nc.sync.dma_start(out=v_sb[:], in_=vidx[:])
            r_sb = cpool.tile([N_ETYPES, D], dt)
            nc.sync.dma_start(out=r_sb[:], in_=relw[:])
            score = cpool.tile([P, t_tot], f32)

            col = 0   # index-column cursor (shared by u/v sides)
            t0 = 0    # tile cursor
            for b in range(NB):
                wu, wv = b // NW, b % NW
                ub = wu * W
                vb = wv * W
                ulen = min(W, N_NODES - ub)
                vlen = min(W, N_NODES - vb)
                for n in _chunks(caps[b]):
                    nt = n // P
                    hu = gpool.tile([P, nt * D], dt, tag="hu")
                    nc.gpsimd.dma_gather(
                        hu[:].rearrange("p (c d) -> p c d", d=D),
                        h_ap[ub : ub + ulen],
                        u_sb[:, col : col + n // 16],
                        n, n, D, elem_step=D,
                        queue_num=q % NQ,
                    )
                    q += 1
                    hv = gpool.tile([P, nt * D], dt, tag="hv")
                    nc.gpsimd.dma_gather(
                        hv[:].rearrange("p (c d) -> p c d", d=D),
                        h_ap[vb : vb + vlen],
                        v_sb[:, col : col + n // 16],
                        n, n, D, elem_step=D,
                        queue_num=q % NQ,
                    )
                    q += 1
                    oh = gpool.tile([N_ETYPES, n], dt, tag="oh")
                    nc.sync.dma_start(
                        out=oh[:], in_=oneh[:, t0 * P : t0 * P + n]
                    )
                    prod = w4pool.tile([P, nt * D], dt, tag="prod")
                    nc.vector.tensor_mul(
                        out=prod[:], in0=hu[:], in1=hv[:]
                    )
                    for j in range(nt):
                        rg = ppool.tile([P, D], f32)
                        nc.tensor.matmul(
                            out=rg[:],
                            lhsT=oh[:, j * P : (j + 1) * P],
                            rhs=r_sb[:],
                            start=True,
                            stop=True,
                        )
                        prod2 = wpool.tile([P, D], dt, tag="prod2")
                        nc.vector.tensor_mul(
                            out=prod2[:],
                            in0=prod[:, j * D : (j + 1) * D],
                            in1=rg[:],
                        )
                        if (t0 + j) % 8 == 0:
                            nc.vector.tensor_reduce(
                                out=score[:, t0 + j : t0 + j + 1],
                                in_=prod2[:],
                                axis=mybir.AxisListType.X,
                                op=mybir.AluOpType.add,
                            )
                        else:
                            nc.scalar.activation(
                                out=prod2[:],
                                in_=prod2[:],
                                func=mybir.ActivationFunctionType.Copy,
                                accum_out=score[:, t0 + j : t0 + j + 1],
                            )
                    col += n // 16
                    t0 += nt

            nc.scalar.activation(
                out=score[:],
                in_=score[:],
                func=mybir.ActivationFunctionType.Sigmoid,
            )
            nc.sync.dma_start(out=out[:], in_=score[:])

    nc.compile()
    return nc


def _get_nc(caps):
    key = (DTYPE, caps)
    if key not in _cache:
        _cache[key] = _build(caps)
    return _cache[key]


def _wrap16(a):
    """[n] int16 -> [128, n/16] wrapped-over-16-partitions, replicated 8x."""
    n = a.shape[0]
    return np.tile(a.reshape(n // 16, 16).T, (8, 1))


def _shard(u32, v32, et):
    """Bucket edges by (u>>15, v>>15) and split each bucket evenly across
    cores. Returns (caps, per-core dict of padded slot arrays, per-core
    edge-id mapping)."""
    key = (u32 >> 15) * NW + (v32 >> 15)
    order = np.argsort(key, kind="stable")
    counts = np.bincount(key, minlength=NB)
    starts = np.concatenate([[0], np.cumsum(counts)])
    n_pc = [(int(c) + N_CORES - 1) // N_CORES for c in counts]
    caps = tuple(max(P, (n + P - 1) // P * P) for n in n_pc)
    tot = sum(caps)

    per_core = []
    for c in range(N_CORES):
        u_slots = np.zeros(tot, np.int32)
        v_slots = np.zeros(tot, np.int32)
        e_slots = np.zeros(tot, np.int64)
        eid = np.full(tot, -1, np.int64)
        pos = 0
        for b in range(NB):
            lo = starts[b] + c * n_pc[b]
            hi = min(starts[b] + (c + 1) * n_pc[b], starts[b + 1])
            if hi > lo:
                ids = order[lo:hi]
                k = hi - lo
                u_slots[pos : pos + k] = u32[ids] - (b // NW) * W
                v_slots[pos : pos + k] = v32[ids] - (b % NW) * W
                e_slots[pos : pos + k] = et[ids]
                eid[pos : pos + k] = ids
            pos += caps[b]
        per_core.append((u_slots, v_slots, e_slots, eid))
    return caps, per_core


def _make_in_maps(h, u, v, etype, rel_weight, caps, per_core):
    np_dt = _np_dt()
    h_c = np.ascontiguousarray(np.asarray(h, np.float32).astype(np_dt))
    rel_c = np.asarray(rel_weight, np.float32).astype(np_dt)

    in_maps = []
    for c in range(N_CORES):
        u_slots, v_slots, e_slots, _eid = per_core[c]
        u_blocks, v_blocks = [], []
        pos = 0
        for b in range(NB):
            for n in _chunks(caps[b]):
                u_blocks.append(_wrap16(u_slots[pos : pos + n].astype(np.int16)))
                v_blocks.append(_wrap16(v_slots[pos : pos + n].astype(np.int16)))
                pos += n
        in_maps.append(
            {
                "h": h_c,
                "uidx": np.ascontiguousarray(np.concatenate(u_blocks, axis=1)),
                "vidx": np.ascontiguousarray(np.concatenate(v_blocks, axis=1)),
                "oneh": np.ascontiguousarray(
                    (e_slots[None, :] == np.arange(N_ETYPES)[:, None]).astype(np_dt)
                ),
                "relw": np.ascontiguousarray(rel_c),
            }
        )
    return in_maps


def run_spmd(h, u, v, etype, rel_weight, trace=False, trace_cores=None):
    """Run the SPMD kernel; returns (full_output, BassKernelResults)."""
    u32 = np.asarray(u, np.int32)
    v32 = np.asarray(v, np.int32)
    et = np.asarray(etype, np.int64)
    n_edges = u32.shape[0]

    caps, per_core = _shard(u32, v32, et)
    nc = _get_nc(caps)
    in_maps = _make_in_maps(h, u, v, etype, rel_weight, caps, per_core)
    res = run_bass_kernel_spmd(
        nc,
        in_maps,
        core_ids=list(range(N_CORES)),
        trace=trace,
        trace_cores=trace_cores,
    )
    result = np.zeros(n_edges, np.float32)
    for c in range(N_CORES):
        o = res.results[c]["out"]            # [P, t_tot] fp32
        vals = o.T.reshape(-1)               # slot-ordered scores
        eid = per_core[c][3]
        m = eid >= 0
        result[eid[m]] = vals[m]
    return result, res


def kernel(h, u, v, etype, rel_weight):
    out, _ = run_spmd(h, u, v, etype, rel_weight)
    return out


# revision 24
# speedup vs baseline: 1.0236x; 1.0236x over previous
"""DistMult edge scoring on 8 Trainium2 NeuronCores.

score[e] = sigmoid(sum_d h[u[e],d] * rel_weight[etype[e],d] * h[v[e],d])

Strategy
--------
Edges are sharded evenly across the 8 cores (pure edge parallelism); h and
rel_weight are replicated per core. The dominant cost is gathering h[u] and
h[v] rows (2 x 250k x D floats), so the kernel is built around the fast
Q7-ucode `dma_gather` (InstDMAGatherAnt):

- int16 gather indices only address 32768 rows, so h is viewed as 4 windows
  of 32768 rows and edges are bucketed by the window pair (u>>15, v>>15).
  Indices are window-relative; each gather instruction reads from its
  window's base AP.
- Every bucket is split evenly across the 8 cores (same per-bucket capacity
  on every core -> one shared SPMD program). Capacities depend on the input,
  so the program is JIT-built per capacity signature and cached.
- 4 SWDGE queues round-robin the gather instructions across the 4 Q7 cpu
  pairs (descriptor generation is the gather bottleneck at ~8.4 ns/row/queue).
- Per gather chunk DVE computes hu*hv for all tiles in one wide fp16
  2x-mode multiply; per 128-edge tile PE expands rel_weight[etype] via a
  one-hot matmul into PSUM and DVE multiplies it in; the free-axis fp32
  reduction is split between DVE (tensor_reduce) and ACT (activation
  accum_out) to balance the engines; ACT applies the sigmoid once at the
  end.
- Tensor data is gathered/multiplied in fp16 (fp32 accumulation): halves the
  gather bytes and doubles DVE throughput at ~1e-3 worst-case relative error.
  Set DTYPE = "float32" for exact mode.
"""

import numpy as np

import concourse.bacc as bacc
import concourse.mybir as mybir
import concourse.tile as tile
from concourse.bass_utils import run_bass_kernel_spmd

N_NODES = 100000
D = 384
N_ETYPES = 8
N_CORES = 8

P = 128
W = 32768                 # int16-addressable h window (rows)
NW = (N_NODES + W - 1) // W   # 4 windows
NB = NW * NW              # 16 (wu, wv) buckets
CH = 1024                 # max gather indices per dma_gather instruction
NQ = 4                    # SWDGE queues

DTYPE = "float16"         # compute/gather dtype: "float16" or "float32"

_cache = {}


def _np_dt():
    return np.float16 if DTYPE == "float16" else np.float32


def _mb_dt():
    return mybir.dt.float16 if DTYPE == "float16" else mybir.dt.float32


def _chunks(cap):
    """Split a bucket capacity (multiple of 128) into gather chunk sizes."""
    out = []
    while cap > 0:
        n = min(cap, CH)
        out.append(n)
        cap -= n
    return out


def _build(caps):
    """Build + compile the SPMD program for per-bucket capacities `caps`
    (tuple of NB ints, each a multiple of 128)."""
    dt = _mb_dt()
    f32 = mybir.dt.float32
    t_tot = sum(caps) // P
    ucols = sum(caps) // 16   # total int16 index columns per side

    nc = bacc.Bacc(
        "TRN2",
        target_bir_lowering=False,
        debug=False,
        enable_asserts=False,
        num_devices=N_CORES,
        num_swdge_queues=NQ,
    )
    h_ap = nc.dram_tensor("h", [N_NODES, D], dt, kind="ExternalInput").ap()
    uidx = nc.dram_tensor("uidx", [P, ucols], mybir.dt.int16, kind="ExternalInput").ap()
    vidx = nc.dram_tensor("vidx", [P, ucols], mybir.dt.int16, kind="ExternalInput").ap()
    oneh = nc.dram_tensor("oneh", [N_ETYPES, t_tot * P], dt, kind="ExternalInput").ap()
    relw = nc.dram_tensor("relw", [N_ETYPES, D], dt, kind="ExternalInput").ap()
    out = nc.dram_tensor("out", [P, t_tot], f32, kind="ExternalOutput").ap()

    q = 0
    with tile.TileContext(nc) as tc:
        with (
            tc.tile_pool(name="const", bufs=1) as cpool,
            tc.tile_pool(name="gath", bufs=6) as gpool,
            tc.tile_pool(name="work", bufs=8) as wpool,
            tc.tile_pool(name="work4", bufs=4) as w4pool,
            tc.tile_pool(name="psum", bufs=6, space="PSUM") as ppool,
        ):
            u_sb = cpool.tile([P, ucols], mybir.dt.int16)
            nc.sync.dma_start(out=u_sb[:], in_=uidx[:])
            v_sb = cpool.tile([P, ucols], mybir.dt.int16)
            nc.sync.dma_start(out=v_sb[:], in_=vidx[:])
            r_sb = cpool.tile([N_ETYPES, D], dt)
            nc.sync.dma_start(out=r_sb[:], in_=relw[:])
            score = cpool.tile([P, t_tot], f32)

            col = 0   # index-column cursor (shared by u/v sides)
            t0 = 0    # tile cursor
            for b in range(NB):
                wu, wv = b // NW, b % NW
                ub = wu * W
                vb = wv * W
                ulen = min(W, N_NODES - ub)
                vlen = min(W, N_NODES - vb)
                for n in _chunks(caps[b]):
                    nt = n // P
                    hu = gpool.tile([P, nt * D], dt, tag="hu")
                    nc.gpsimd.dma_gather(
                        hu[:].rearrange("p (c d) -> p c d", d=D),
                        h_ap[ub : ub + ulen],
                        u_sb[:, col : col + n // 16],
                        n, n, D, elem_step=D,
                        queue_num=q % NQ,
                    )
                    q += 1
                    hv = gpool.tile([P, nt * D], dt, tag="hv")
                    nc.gpsimd.dma_gather(
                        hv[:].rearrange("p (c d) -> p c d", d=D),
                        h_ap[vb : vb + vlen],
                        v_sb[:, col : col + n // 16],
                        n, n, D, elem_step=D,
                        queue_num=q % NQ,
                    )
                    q += 1
                    oh = gpool.tile([N_ETYPES, n], dt, tag="oh")
                    nc.sync.dma_start(
                        out=oh[:], in_=oneh[:, t0 * P : t0 * P + n]
                    )
                    prod = w4pool.tile([P, nt * D], dt, tag="prod")
                    nc.vector.tensor_mul(
                        out=prod[:], in0=hu[:], in1=hv[:]
                    )
                    for j in range(nt):
                        rg = ppool.tile([P, D], f32)
                        nc.tensor.matmul(
                            out=rg[:],
                            lhsT=oh[:, j * P : (j + 1) * P],
                            rhs=r_sb[:],
                            start=True,
                            stop=True,
                        )
                        prod2 = wpool.tile([P, D], dt, tag="prod2")
                        nc.vector.tensor_mul(
                            out=prod2[:],
                            in0=prod[:, j * D : (j + 1) * D],
                            in1=rg[:],
                        )
                        if (t0 + j) % 8 == 0:
                            nc.vector.tensor_reduce(
                                out=score[:, t0 + j : t0 + j + 1],
                                in_=prod2[:],
                                axis=mybir.AxisListType.X,
                                op=mybir.AluOpType.add,
                            )
                        else:
                            nc.scalar.activation(
                                out=prod2[:],
                                in_=prod2[:],
                                func=mybir.ActivationFunctionType.Copy,
                                accum_out=score[:, t0 + j : t0 + j + 1],
                            )
                    col += n // 16
                    t0 += nt

            nc.scalar.activation(
                out=score[:],
                in_=score[:],
                func=mybir.ActivationFunctionType.Sigmoid,
            )
            nc.sync.dma_start(out=out[:], in_=score[:])

    nc.compile()
    return nc


def _get_nc(caps):
    key = (DTYPE, caps)
    if key not in _cache:
        _cache[key] = _build(caps)
    return _cache[key]


def _wrap16(a):
    """[n] int16 -> [128, n/16] wrapped-over-16-partitions, replicated 8x."""
    n = a.shape[0]
    return np.tile(a.reshape(n // 16, 16).T, (8, 1))


def _shard(u32, v32, et):
    """Bucket edges by (u>>15, v>>15) and split each bucket evenly across
    cores. Returns (caps, per-core dict of padded slot arrays, per-core
    edge-id mapping)."""
    key = (u32 >> 15) * NW + (v32 >> 15)
    order = np.argsort(key, kind="stable")
    counts = np.bincount(key, minlength=NB)
    starts = np.concatenate([[0], np.cumsum(counts)])
    n_pc = [(int(c) + N_CORES - 1) // N_CORES for c in counts]
    caps = tuple(max(P, (n + P - 1) // P * P) for n in n_pc)
    tot = sum(caps)

    per_core = []
    for c in range(N_CORES):
        u_slots = np.zeros(tot, np.int32)
        v_slots = np.zeros(tot, np.int32)
        e_slots = np.zeros(tot, np.int64)
        eid = np.full(tot, -1, np.int64)
        pos = 0
        for b in range(NB):
            lo = starts[b] + c * n_pc[b]
            hi = min(starts[b] + (c + 1) * n_pc[b], starts[b + 1])
            if hi > lo:
                ids = order[lo:hi]
                k = hi - lo
                u_slots[pos : pos + k] = u32[ids] - (b // NW) * W
                v_slots[pos : pos + k] = v32[ids] - (b % NW) * W
                e_slots[pos : pos + k] = et[ids]
                eid[pos : pos + k] = ids
            pos += caps[b]
        per_core.append((u_slots, v_slots, e_slots, eid))
    return caps, per_core


def _make_in_maps(h, u, v, etype, rel_weight, caps, per_core):
    np_dt = _np_dt()
    h_c = np.ascontiguousarray(np.asarray(h, np.float32).astype(np_dt))
    rel_c = np.asarray(rel_weight, np.float32).astype(np_dt)

    in_maps = []
    for c in range(N_CORES):
        u_slots, v_slots, e_slots, _eid = per_core[c]
        u_blocks, v_blocks = [], []
        pos = 0
        for b in range(NB):
            for n in _chunks(caps[b]):
                u_blocks.append(_wrap16(u_slots[pos : pos + n].astype(np.int16)))
                v_blocks.append(_wrap16(v_slots[pos : pos + n].astype(np.int16)))
                pos += n
        in_maps.append(
            {
                "h": h_c,
                "uidx": np.ascontiguousarray(np.concatenate(u_blocks, axis=1)),
                "vidx": np.ascontiguousarray(np.concatenate(v_blocks, axis=1)),
                "oneh": np.ascontiguousarray(
                    (e_slots[None, :] == np.arange(N_ETYPES)[:, None]).astype(np_dt)
                ),
                "relw": np.ascontiguousarray(rel_c),
            }
        )
    return in_maps


def run_spmd(h, u, v, etype, rel_weight, trace=False, trace_cores=None):
    """Run the SPMD kernel; returns (full_output, BassKernelResults)."""
    u32 = np.asarray(u, np.int32)
    v32 = np.asarray(v, np.int32)
    et = np.asarray(etype, np.int64)
    n_edges = u32.shape[0]

    caps, per_core = _shard(u32, v32, et)
    nc = _get_nc(caps)
    in_maps = _make_in_maps(h, u, v, etype, rel_weight, caps, per_core)
    res = run_bass_kernel_spmd(
        nc,
        in_maps,
        core_ids=list(range(N_CORES)),
        trace=trace,
        trace_cores=trace_cores,
    )
    result = np.zeros(n_edges, np.float32)
    for c in range(N_CORES):
        o = res.results[c]["out"]            # [P, t_tot] fp32
        vals = o.T.reshape(-1)               # slot-ordered scores
        eid = per_core[c][3]
        m = eid >= 0
        result[eid[m]] = vals[m]
    return result, res


def kernel(h, u, v, etype, rel_weight):
    out, _ = run_spmd(h, u, v, etype, rel_weight)
    return out
